# revision 3
# baseline (speedup 1.0000x reference)
"""TRN2 Bass/Tile kernel: graph neural ODE integrated with RK4.

Computes pred_y[t] for t=0..19 where
    dx/dt = f(x) = tanh((edge @ x) @ W1 + x @ W2 + b)
via 19 RK4 steps from x0 = node, data-parallel over the batch axis:
16 batches sharded 2-per-core across 8 NeuronCores (SPMD, no collectives).

Layout strategy (per core, per batch):
  - State lives TRANSPOSED in SBUF: xT[d, i]  (feature on partitions,
    512 nodes on the free axis).
  - v-stage:  v = x @ W1 in natural [node, feat] layout via
        matmul(lhsT=xT[:, c*128:(c+1)*128], rhs=[W1|W2])  -> psum[j, 256]
    (the widened [W1|W2] moving operand keeps the fp32r fast path,
    which needs a moving free dim >= 256; the W2 half is discarded).
  - z-stage:  zT[e, i] = (edge @ v)^T + (x @ W2)^T accumulated in one
    PSUM bank:
        matmul(lhsT=W2, rhs=xT) + sum_c matmul(lhsT=v_nat[:, c], rhs=edgeT[c])
  - tanh on ScalarE straight out of PSUM with per-partition bias b.
  - RK4 state algebra on VectorE with fused scalar_tensor_tensor ops.
  - edge is consumed transposed (edgeT[j, i]); the host pre-transposes it
    (free), and the per-step outputs are written transposed [d, i] and
    un-transposed on the host (also free).

All matmuls run in float32r (full fp32 storage; PE runs the fast 1
cycle/row mode for moving dims >= 256).
"""

import numpy as np

import concourse.bass as bass
import concourse.tile as tile
from concourse import bacc, mybir
from concourse import bass_utils

B, N, D, T = 16, 512, 128, 20
NCORES = 8
BPC = B // NCORES  # batches per core

F32 = mybir.dt.float32
F32R = mybir.dt.float32r
ALU = mybir.AluOpType
ACTF = mybir.ActivationFunctionType


def build_program(dts):
    """Build the SPMD Bass program (identical on all cores)."""
    nc = bacc.Bacc(
        "TRN2",
        target_bir_lowering=False,
        debug=False,
        num_devices=NCORES,
    )
    nodeT_in = nc.dram_tensor("nodeT", [BPC, D, N], F32R, kind="ExternalInput").ap()
    edgeT_in = nc.dram_tensor("edgeT", [BPC, N, N], F32R, kind="ExternalInput").ap()
    wcat_in = nc.dram_tensor("wcat", [D, 2 * D], F32R, kind="ExternalInput").ap()
    w2_in = nc.dram_tensor("w2", [D, D], F32R, kind="ExternalInput").ap()
    b_in = nc.dram_tensor("bvec", [D, 1], F32, kind="ExternalInput").ap()
    out_t = nc.dram_tensor("out", [T - 1, BPC, D, N], F32R, kind="ExternalOutput").ap()

    with tile.TileContext(nc) as tc:
        _emit(tc, nodeT_in, edgeT_in, wcat_in, w2_in, b_in, out_t, dts)
    nc.compile()
    return nc


def _emit(tc, nodeT_in, edgeT_in, wcat_in, w2_in, b_in, out_t, dts):
    from contextlib import ExitStack

    nc = tc.nc
    with ExitStack() as ctx:
        const = ctx.enter_context(tc.tile_pool(name="const", bufs=1))
        state = ctx.enter_context(tc.tile_pool(name="state", bufs=2))
        kpool = ctx.enter_context(tc.tile_pool(name="k", bufs=2))
        vpool = ctx.enter_context(tc.tile_pool(name="v", bufs=2))
        tmp = ctx.enter_context(tc.tile_pool(name="tmp", bufs=2))
        pv = ctx.enter_context(tc.tile_pool(name="pv", bufs=1, space="PSUM"))
        pz = ctx.enter_context(tc.tile_pool(name="pz", bufs=1, space="PSUM"))

        wcat = const.tile([D, 2 * D], F32R, tag="wcat")
        nc.sync.dma_start(wcat[:], wcat_in)
        w2 = const.tile([D, D], F32R, tag="w2")
        nc.sync.dma_start(w2[:], w2_in)
        bias = const.tile([D, 1], F32, tag="bias")
        nc.sync.dma_start(bias[:], b_in)

        edge_sb = []
        x_cur = []
        for bb in range(BPC):
            et = const.tile([128, 4 * N], F32R, tag=f"edge{bb}")
            for c in range(4):
                nc.sync.dma_start(
                    et[:, c * N : (c + 1) * N],
                    edgeT_in[bb, c * 128 : (c + 1) * 128, :],
                )
            edge_sb.append(et)
            x0 = state.tile([D, N], F32R, tag=f"x{bb}")
            nc.sync.dma_start(x0[:], nodeT_in[bb])
            x_cur.append(x0)

        def emit_eval(bb, y, k_out):
            """k_out = tanh((E @ (y^T @ W1) + y^T @ W2 + b))^T, y = state^T."""
            pvt = pv.tile([128, 4 * 256], F32, tag=f"pv{bb}")
            for c in range(4):
                nc.tensor.matmul(
                    pvt[:, c * 256 : (c + 1) * 256],
                    lhsT=y[:, c * 128 : (c + 1) * 128],
                    rhs=wcat[:],
                    start=True,
                    stop=True,
                )
            vt = vpool.tile([128, N], F32R, tag=f"v{bb}")
            nc.scalar.activation(
                vt[:].rearrange("p (c e) -> p c e", c=4),
                pvt[:].rearrange("p (c w) -> p c w", c=4)[:, :, 0:128],
                ACTF.Copy,
            )
            pzt = pz.tile([128, N], F32, tag=f"pz{bb}")
            nc.tensor.matmul(
                pzt[:],
                lhsT=w2[:],
                rhs=y[:],
                start=True,
                stop=False,
            )
            for c in range(4):
                nc.tensor.matmul(
                    pzt[:],
                    lhsT=vt[:, c * 128 : (c + 1) * 128],
                    rhs=edge_sb[bb][:, c * N : (c + 1) * N],
                    start=False,
                    stop=(c == 3),
                )
            nc.scalar.activation(k_out[:], pzt[:], ACTF.Tanh, bias=bias[:])

        for t in range(T - 1):
            dt = float(dts[t])
            k1s, k2s, k3s, k4s = [], [], [], []
            # k1
            for bb in range(BPC):
                k1 = kpool.tile([D, N], F32, tag=f"k1_{bb}")
                emit_eval(bb, x_cur[bb], k1)
                k1s.append(k1)
            # x2 = x + dt/2 k1 ; k2
            xi = []
            for bb in range(BPC):
                x2 = tmp.tile([D, N], F32R, tag=f"xi{bb}")
                nc.vector.scalar_tensor_tensor(
                    x2[:], k1s[bb][:], dt / 2, x_cur[bb][:], ALU.mult, ALU.add
                )
                xi.append(x2)
            for bb in range(BPC):
                k2 = kpool.tile([D, N], F32, tag=f"k2_{bb}")
                emit_eval(bb, xi[bb], k2)
                k2s.append(k2)
            # x3 = x + dt/2 k2 ; k3
            xi = []
            for bb in range(BPC):
                x3 = tmp.tile([D, N], F32R, tag=f"xi{bb}")
                nc.vector.scalar_tensor_tensor(
                    x3[:], k2s[bb][:], dt / 2, x_cur[bb][:], ALU.mult, ALU.add
                )
                xi.append(x3)
            for bb in range(BPC):
                k3 = kpool.tile([D, N], F32, tag=f"k3_{bb}")
                emit_eval(bb, xi[bb], k3)
                k3s.append(k3)
            # x4 = x + dt k3 ; k4
            xi = []
            for bb in range(BPC):
                x4 = tmp.tile([D, N], F32R, tag=f"xi{bb}")
                nc.vector.scalar_tensor_tensor(
                    x4[:], k3s[bb][:], dt, x_cur[bb][:], ALU.mult, ALU.add
                )
                xi.append(x4)
            for bb in range(BPC):
                k4 = kpool.tile([D, N], F32, tag=f"k4_{bb}")
                emit_eval(bb, xi[bb], k4)
                k4s.append(k4)
            # x_new = x + dt/6 (k1 + 2 k2 + 2 k3 + k4)
            for bb in range(BPC):
                s1 = tmp.tile([D, N], F32, tag=f"s1{bb}")
                nc.vector.scalar_tensor_tensor(
                    s1[:], k2s[bb][:], 2.0, k1s[bb][:], ALU.mult, ALU.add
                )
                s2 = tmp.tile([D, N], F32, tag=f"s2{bb}")
                nc.vector.scalar_tensor_tensor(
                    s2[:], k3s[bb][:], 2.0, k4s[bb][:], ALU.mult, ALU.add
                )
                s3 = tmp.tile([D, N], F32, tag=f"s3{bb}")
                nc.vector.tensor_add(s3[:], s1[:], s2[:])
                x_new = state.tile([D, N], F32R, tag=f"x{bb}")
                nc.vector.scalar_tensor_tensor(
                    x_new[:], s3[:], dt / 6.0, x_cur[bb][:], ALU.mult, ALU.add
                )
                nc.sync.dma_start(out_t[t, bb], x_new[:])
                x_cur[bb] = x_new


def round_f32r(x):
    """Round fp32 values to the fp32r subset (11 explicit mantissa bits,
    low 12 bits zero) with round-to-nearest-even — matches what the PE
    consumes in fp32r mode, so host-side rounding keeps hardware exact."""
    u = np.ascontiguousarray(x, dtype=np.float32).view(np.uint32)
    u = (u + 0x7FF + ((u >> 12) & 1)) & np.uint32(0xFFFFF000)
    return u.view(np.float32)


def make_in_maps(node, edge, W1, W2, b):
    wcat = round_f32r(np.concatenate([W1, W2], axis=1))
    w2c = round_f32r(W2)
    bc = np.ascontiguousarray(np.reshape(b, (D, 1)), dtype=np.float32)
    in_maps = []
    for core in range(NCORES):
        sl = slice(core * BPC, (core + 1) * BPC)
        in_maps.append(
            {
                "nodeT": round_f32r(node[sl].transpose(0, 2, 1)),
                "edgeT": round_f32r(edge[sl].transpose(0, 2, 1)),
                "wcat": wcat,
                "w2": w2c,
                "bvec": bc,
            }
        )
    return in_maps


LAST_RESULT = None


def kernel(node, edge, time_steps, W1, W2, b, trace=False):
    node = np.asarray(node, dtype=np.float32)
    edge = np.asarray(edge, dtype=np.float32)
    time_steps = np.asarray(time_steps, dtype=np.float32)
    W1 = np.asarray(W1, dtype=np.float32)
    W2 = np.asarray(W2, dtype=np.float32)
    b = np.asarray(b, dtype=np.float32)

    dts = time_steps[1:] - time_steps[:-1]
    nc = build_program(dts)
    in_maps = make_in_maps(node, edge, W1, W2, b)
    res = bass_utils.run_bass_kernel_spmd(
        nc, in_maps, core_ids=list(range(NCORES)), trace=trace
    )
    global LAST_RESULT
    LAST_RESULT = res
    outs = [res.results[c]["out"] for c in range(NCORES)]  # [T-1, BPC, D, N]
    full = np.concatenate(outs, axis=1)  # [T-1, B, D, N]
    pred = np.empty((T, B, N, D), dtype=np.float32)
    pred[0] = node
    pred[1:] = full.transpose(0, 1, 3, 2)
    return pred


# revision 6
# speedup vs baseline: 1.5480x; 1.5480x over previous
"""TRN2 Bass/Tile kernel: graph neural ODE integrated with RK4.

Computes pred_y[t] for t=0..19 where
    dx/dt = f(x) = tanh((edge @ x) @ W1 + x @ W2 + b)
via 19 RK4 steps from x0 = node, data-parallel over the batch axis:
16 batches sharded 2-per-core across 8 NeuronCores (SPMD, no collectives).

Layout strategy (per core, per batch):
  - State lives TRANSPOSED in SBUF: xT[d, i]  (feature on partitions,
    512 nodes on the free axis).
  - v-stage:  v = y @ W1 in natural [node, feat] layout via
        matmul(lhsT=yT[:, c*128:(c+1)*128], rhs=[W1|W2])  -> psum[j, 256]
    (the widened [W1|W2] moving operand keeps the fp32r fast path,
    which needs a moving free dim >= 256; the W2 half is discarded).
  - z-stage:  zT[e, i] = (edge @ v)^T + (y @ W2)^T accumulated in one
    PSUM bank.
  - Z-LINEARITY: z() is linear, so the RK4 intermediate states never
    materialize:  z(x + c*k) = Z1 + c*Z(k).  Evals 2-4 run the v/z
    stages on k_{i-1} with pre-scaled weights (c*[W1|W2], c*W2) and
    accumulate on top of a seed matmul(identity, Z1_sb).
  - tanh on ScalarE straight out of PSUM with per-partition bias b.
  - RK4 combine is a chain of fused scalar_tensor_tensor ops
        a1 = x + dt/6 k1; a2 = a1 + dt/3 k2; a3 = a2 + dt/3 k3;
        x_new = a3 + dt/6 k4
    where a_i runs as soon as k_i exists (off the critical path).
  - edge is consumed transposed (edgeT[j, i]); the host pre-transposes it
    (free), and the per-step outputs are written transposed [d, i] and
    un-transposed on the host (also free).

All matmuls run in float32r (fp32 rounded to 11 explicit mantissa bits;
the PE runs 1 cycle/row for moving dims >= 256).  Values feeding matmuls
are produced as float32r (DMA of host-pre-rounded data, ACT tanh/copy
outputs, DVE STT outputs), which is what the walrus verifier requires.
"""

import numpy as np

import concourse.bass as bass
import concourse.tile as tile
from concourse import bacc, mybir
from concourse import bass_utils

B, N, D, T = 16, 512, 128, 20
NCORES = 8
BPC = B // NCORES  # batches per core

F32 = mybir.dt.float32
F32R = mybir.dt.float32r
ALU = mybir.AluOpType
ACTF = mybir.ActivationFunctionType


def build_program(dts, repeat=1):
    """Build the SPMD Bass program (identical on all cores).

    repeat > 1 re-runs the whole integration from x0 that many times
    (timing runs only; the output stays that of the final pass).
    """
    nc = bacc.Bacc(
        "TRN2",
        target_bir_lowering=False,
        debug=False,
        num_devices=NCORES,
    )
    dt_vals = sorted({float(d) for d in dts})
    nodeT_in = nc.dram_tensor("nodeT", [BPC, D, N], F32R, kind="ExternalInput").ap()
    edgeT_in = nc.dram_tensor("edgeT", [BPC, N, N], F32R, kind="ExternalInput").ap()
    # per distinct dt: [W1|W2], c/2*[W1|W2], c*[W1|W2] are slices of wcats
    wcats_in = nc.dram_tensor(
        "wcats", [1 + 2 * len(dt_vals), D, 2 * D], F32R, kind="ExternalInput"
    ).ap()
    w2s_in = nc.dram_tensor(
        "w2s", [1 + 2 * len(dt_vals), D, D], F32R, kind="ExternalInput"
    ).ap()
    ident_in = nc.dram_tensor("ident", [D, D], F32R, kind="ExternalInput").ap()
    b_in = nc.dram_tensor("bvec", [D, 1], F32, kind="ExternalInput").ap()
    out_t = nc.dram_tensor("out", [T - 1, BPC, D, N], F32R, kind="ExternalOutput").ap()

    with tile.TileContext(nc) as tc:
        _emit(
            tc, nodeT_in, edgeT_in, wcats_in, w2s_in, ident_in, b_in, out_t,
            dts, dt_vals, repeat,
        )
    nc.compile()
    return nc


def _emit(tc, nodeT_in, edgeT_in, wcats_in, w2s_in, ident_in, b_in, out_t,
          dts, dt_vals, repeat):
    from contextlib import ExitStack

    nc = tc.nc
    nw = 1 + 2 * len(dt_vals)
    with ExitStack() as ctx:
        const = ctx.enter_context(tc.tile_pool(name="const", bufs=1))
        state = ctx.enter_context(tc.tile_pool(name="state", bufs=2))
        kpool = ctx.enter_context(tc.tile_pool(name="k", bufs=2))
        vpool = ctx.enter_context(tc.tile_pool(name="v", bufs=2))
        zpool = ctx.enter_context(tc.tile_pool(name="z1", bufs=2))
        tmp = ctx.enter_context(tc.tile_pool(name="tmp", bufs=2))
        pv = ctx.enter_context(tc.tile_pool(name="pv", bufs=1, space="PSUM"))
        pz = ctx.enter_context(tc.tile_pool(name="pz", bufs=1, space="PSUM"))

        wcats = const.tile([D, nw * 2 * D], F32R, tag="wcats")
        for w in range(nw):
            nc.sync.dma_start(wcats[:, w * 2 * D : (w + 1) * 2 * D], wcats_in[w])
        w2s = const.tile([D, nw * D], F32R, tag="w2s")
        for w in range(nw):
            nc.sync.dma_start(w2s[:, w * D : (w + 1) * D], w2s_in[w])
        ident = const.tile([D, D], F32R, tag="ident")
        nc.sync.dma_start(ident[:], ident_in)
        bias = const.tile([D, 1], F32, tag="bias")
        nc.sync.dma_start(bias[:], b_in)

        def wcat_slice(idx):
            return wcats[:, idx * 2 * D : (idx + 1) * 2 * D]

        def w2_slice(idx):
            return w2s[:, idx * D : (idx + 1) * D]

        edge_sb = []
        for bb in range(BPC):
            et = const.tile([128, 4 * N], F32R, tag=f"edge{bb}")
            for c in range(4):
                nc.sync.dma_start(
                    et[:, c * N : (c + 1) * N],
                    edgeT_in[bb, c * 128 : (c + 1) * 128, :],
                )
            edge_sb.append(et)

        def emit_vstage(bb, y, widx):
            """psum v-tile: [x@(c W1) | x@(c W2)] per 128-node chunk."""
            pvt = pv.tile([128, 4 * 256], F32, tag=f"pv{bb}")
            for c in range(4):
                nc.tensor.matmul(
                    pvt[:, c * 256 : (c + 1) * 256],
                    lhsT=y[:, c * 128 : (c + 1) * 128],
                    rhs=wcat_slice(widx),
                    start=True,
                    stop=True,
                )
            return pvt

        def emit_vcopy(bb, pvt):
            vt = vpool.tile([128, N], F32R, tag=f"v{bb}")
            dst = vt[:].rearrange("p (c e) -> p c e", c=4)
            src = pvt[:].rearrange("p (c w) -> p c w", c=4)[:, :, 0:128]
            nc.scalar.activation(dst, src, ACTF.Copy)
            return vt

        def emit_zstage(bb, y, vt, widx, seed_sb):
            """psum z = [seed] + (E @ v)^T + (y^T @ c W2)^T."""
            pzt = pz.tile([128, N], F32, tag=f"pz{bb}")
            if seed_sb is not None:
                nc.tensor.matmul(
                    pzt[:], lhsT=ident[:], rhs=seed_sb[:], start=True, stop=False
                )
            nc.tensor.matmul(
                pzt[:],
                lhsT=w2_slice(widx),
                rhs=y[:],
                start=(seed_sb is None),
                stop=False,
            )
            for c in range(4):
                nc.tensor.matmul(
                    pzt[:],
                    lhsT=vt[:, c * 128 : (c + 1) * 128],
                    rhs=edge_sb[bb][:, c * N : (c + 1) * N],
                    start=False,
                    stop=(c == 3),
                )
            return pzt

        for rep in range(repeat):
            x_cur = []
            for bb in range(BPC):
                x0 = state.tile([D, N], F32R, tag=f"x{bb}")
                nc.sync.dma_start(x0[:], nodeT_in[bb])
                x_cur.append(x0)

            for t in range(T - 1):
                dt = float(dts[t])
                di = dt_vals.index(dt)
                w_half = 1 + 2 * di      # (dt/2) * [W1|W2]
                w_full_dt = 2 + 2 * di   # dt * [W1|W2]
                ks = [[None] * 4 for _ in range(BPC)]
                acc = [None] * BPC
                z1_sb = [None] * BPC
                for e in range(4):
                    widx = (0, w_half, w_half, w_full_dt)[e]
                    ys = [
                        x_cur[bb] if e == 0 else ks[bb][e - 1] for bb in range(BPC)
                    ]
                    pvts = [emit_vstage(bb, ys[bb], widx) for bb in range(BPC)]
                    vts = [emit_vcopy(bb, pvts[bb]) for bb in range(BPC)]
                    pzts = [
                        emit_zstage(
                            bb, ys[bb], vts[bb], widx,
                            None if e == 0 else z1_sb[bb],
                        )
                        for bb in range(BPC)
                    ]
                    for bb in range(BPC):
                        k = kpool.tile([D, N], F32R, tag=f"k{e}_{bb}")
                        nc.scalar.activation(k[:], pzts[bb][:], ACTF.Tanh, bias=bias[:])
                        ks[bb][e] = k
                    if e == 0:
                        for bb in range(BPC):
                            z1 = zpool.tile([D, N], F32R, tag=f"z1_{bb}")
                            nc.vector.tensor_copy(z1[:], pzts[bb][:])
                            z1_sb[bb] = z1
                    # RK4 combine chain, one link per eval (off critical path)
                    cscale = (dt / 6.0, dt / 3.0, dt / 3.0, dt / 6.0)[e]
                    for bb in range(BPC):
                        prev = x_cur[bb] if e == 0 else acc[bb]
                        if e < 3:
                            a = tmp.tile([D, N], F32, tag=f"a{bb}")
                            nc.vector.scalar_tensor_tensor(
                                a[:], ks[bb][e][:], cscale, prev[:], ALU.mult, ALU.add
                            )
                            acc[bb] = a
                        else:
                            x_new = state.tile([D, N], F32R, tag=f"x{bb}")
                            nc.vector.scalar_tensor_tensor(
                                x_new[:], ks[bb][e][:], cscale, prev[:],
                                ALU.mult, ALU.add,
                            )
                            if rep == repeat - 1:
                                nc.sync.dma_start(out_t[t, bb], x_new[:])
                            x_cur[bb] = x_new


def round_f32r(x):
    """Round fp32 values to the fp32r subset (11 explicit mantissa bits,
    low 12 bits zero) with round-to-nearest-even — matches what the PE
    consumes in fp32r mode, so host-side rounding keeps hardware exact."""
    u = np.ascontiguousarray(x, dtype=np.float32).view(np.uint32)
    u = (u + 0x7FF + ((u >> 12) & 1)) & np.uint32(0xFFFFF000)
    return u.view(np.float32)


def make_in_maps(node, edge, time_steps, W1, W2, b):
    dts = np.asarray(time_steps, np.float32)
    dts = dts[1:] - dts[:-1]
    dt_vals = sorted({float(d) for d in dts})
    wcat = np.concatenate([W1, W2], axis=1).astype(np.float32)
    wcats = [wcat]
    w2s = [W2.astype(np.float32)]
    for dv in dt_vals:
        wcats.append(wcat * (dv / 2))
        wcats.append(wcat * dv)
        w2s.append(W2 * (dv / 2))
        w2s.append(W2 * dv)
    wcats = round_f32r(np.stack(wcats))
    w2s = round_f32r(np.stack(w2s))
    ident = round_f32r(np.eye(D, dtype=np.float32))
    bc = np.ascontiguousarray(np.reshape(b, (D, 1)), dtype=np.float32)
    in_maps = []
    for core in range(NCORES):
        sl = slice(core * BPC, (core + 1) * BPC)
        in_maps.append(
            {
                "nodeT": round_f32r(node[sl].transpose(0, 2, 1)),
                "edgeT": round_f32r(edge[sl].transpose(0, 2, 1)),
                "wcats": wcats,
                "w2s": w2s,
                "ident": ident,
                "bvec": bc,
            }
        )
    return in_maps


LAST_RESULT = None


def kernel(node, edge, time_steps, W1, W2, b, trace=False):
    node = np.asarray(node, dtype=np.float32)
    edge = np.asarray(edge, dtype=np.float32)
    time_steps = np.asarray(time_steps, dtype=np.float32)
    W1 = np.asarray(W1, dtype=np.float32)
    W2 = np.asarray(W2, dtype=np.float32)
    b = np.asarray(b, dtype=np.float32)

    dts = time_steps[1:] - time_steps[:-1]
    nc = build_program(dts)
    in_maps = make_in_maps(node, edge, time_steps, W1, W2, b)
    res = bass_utils.run_bass_kernel_spmd(
        nc, in_maps, core_ids=list(range(NCORES)), trace=trace
    )
    global LAST_RESULT
    LAST_RESULT = res
    outs = [res.results[c]["out"] for c in range(NCORES)]  # [T-1, BPC, D, N]
    full = np.concatenate(outs, axis=1)  # [T-1, B, D, N]
    pred = np.empty((T, B, N, D), dtype=np.float32)
    pred[0] = node
    pred[1:] = full.transpose(0, 1, 3, 2)
    return pred


# revision 8
# speedup vs baseline: 172.6948x; 111.5605x over previous
"""TRN2 Bass/Tile kernel: graph neural ODE integrated with RK4.

Computes pred_y[t] for t=0..19 where
    dx/dt = f(x) = tanh((edge @ x) @ W1 + x @ W2 + b)
via 19 RK4 steps from x0 = node, data-parallel over the batch axis:
16 batches sharded 2-per-core across 8 NeuronCores (SPMD, no collectives).

Layout strategy (per core, per batch):
  - State lives TRANSPOSED in SBUF: xT[d, i]  (feature on partitions,
    512 nodes on the free axis).
  - v-stage:  v = y @ W1 in natural [node, feat] layout via
        matmul(lhsT=yT[:, c*128:(c+1)*128], rhs=[W1|W2])  -> psum[j, 256]
    (the widened [W1|W2] moving operand keeps the fp32r fast path,
    which needs a moving free dim >= 256; the W2 half is discarded).
  - z-stage:  zT[e, i] = (edge @ v)^T + (y @ W2)^T accumulated in one
    PSUM bank.
  - Z-LINEARITY: z() is linear, so the RK4 intermediate states never
    materialize:  z(x + c*k) = Z1 + c*Z(k).  Evals 2-4 run the v/z
    stages on k_{i-1} with pre-scaled weights (c*[W1|W2], c*W2) and
    accumulate on top of a seed matmul(identity, Z1_sb).
  - tanh on ScalarE straight out of PSUM with per-partition bias b.
  - RK4 combine is a chain of fused scalar_tensor_tensor ops
        a1 = x + dt/6 k1; a2 = a1 + dt/3 k2; a3 = a2 + dt/3 k3;
        x_new = a3 + dt/6 k4
    where a_i runs as soon as k_i exists (off the critical path).
  - edge is consumed transposed (edgeT[j, i]); the host pre-transposes it
    (free), and the per-step outputs are written transposed [d, i] and
    un-transposed on the host (also free).

All matmuls run in float32r (fp32 rounded to 11 explicit mantissa bits;
the PE runs 1 cycle/row for moving dims >= 256).  Values feeding matmuls
are produced as float32r (DMA of host-pre-rounded data, ACT tanh/copy
outputs, DVE STT outputs), which is what the walrus verifier requires.
"""

import numpy as np

import concourse.bass as bass
import concourse.tile as tile
from concourse import bacc, mybir
from concourse import bass_utils

B, N, D, T = 16, 512, 128, 20
NCORES = 8
BPC = B // NCORES  # batches per core

F32 = mybir.dt.float32
F32R = mybir.dt.float32r
ALU = mybir.AluOpType
ACTF = mybir.ActivationFunctionType


def build_program(dts, repeat=1):
    """Build the SPMD Bass program (identical on all cores).

    repeat > 1 re-runs the whole integration from x0 that many times
    (timing runs only; the output stays that of the final pass).
    """
    nc = bacc.Bacc(
        "TRN2",
        target_bir_lowering=False,
        debug=False,
        num_devices=NCORES,
    )
    dt_vals = sorted({float(d) for d in dts})
    nodeT_in = nc.dram_tensor("nodeT", [BPC, D, N], F32R, kind="ExternalInput").ap()
    edgeT_in = nc.dram_tensor("edgeT", [BPC, N, N], F32R, kind="ExternalInput").ap()
    # per distinct dt: [W1|W2], c/2*[W1|W2], c*[W1|W2] are slices of wcats
    wcats_in = nc.dram_tensor(
        "wcats", [1 + 2 * len(dt_vals), D, 2 * D], F32R, kind="ExternalInput"
    ).ap()
    w2s_in = nc.dram_tensor(
        "w2s", [1 + 2 * len(dt_vals), D, D], F32R, kind="ExternalInput"
    ).ap()
    ident_in = nc.dram_tensor("ident", [D, D], F32R, kind="ExternalInput").ap()
    b_in = nc.dram_tensor("bvec", [D, 1], F32, kind="ExternalInput").ap()
    out_t = nc.dram_tensor("out", [T - 1, BPC, D, N], F32R, kind="ExternalOutput").ap()

    with tile.TileContext(nc) as tc:
        _emit(
            tc, nodeT_in, edgeT_in, wcats_in, w2s_in, ident_in, b_in, out_t,
            dts, dt_vals, repeat,
        )
    nc.compile()
    return nc


def _emit(tc, nodeT_in, edgeT_in, wcats_in, w2s_in, ident_in, b_in, out_t,
          dts, dt_vals, repeat):
    from contextlib import ExitStack

    nc = tc.nc
    nw = 1 + 2 * len(dt_vals)
    with ExitStack() as ctx:
        const = ctx.enter_context(tc.tile_pool(name="const", bufs=1))
        state = ctx.enter_context(tc.tile_pool(name="state", bufs=2))
        kpool = ctx.enter_context(tc.tile_pool(name="k", bufs=2))
        vpool = ctx.enter_context(tc.tile_pool(name="v", bufs=2))
        zpool = ctx.enter_context(tc.tile_pool(name="z1", bufs=2))
        tmp = ctx.enter_context(tc.tile_pool(name="tmp", bufs=2))
        pv = ctx.enter_context(tc.tile_pool(name="pv", bufs=1, space="PSUM"))
        pz = ctx.enter_context(tc.tile_pool(name="pz", bufs=1, space="PSUM"))

        wcats = const.tile([D, nw * 2 * D], F32R, tag="wcats")
        for w in range(nw):
            nc.sync.dma_start(wcats[:, w * 2 * D : (w + 1) * 2 * D], wcats_in[w])
        w2s = const.tile([D, nw * D], F32R, tag="w2s")
        for w in range(nw):
            nc.sync.dma_start(w2s[:, w * D : (w + 1) * D], w2s_in[w])
        ident = const.tile([D, D], F32R, tag="ident")
        nc.sync.dma_start(ident[:], ident_in)
        bias = const.tile([D, 1], F32, tag="bias")
        nc.sync.dma_start(bias[:], b_in)

        def wcat_slice(idx):
            return wcats[:, idx * 2 * D : (idx + 1) * 2 * D]

        def w2_slice(idx):
            return w2s[:, idx * D : (idx + 1) * D]

        edge_sb = []
        for bb in range(BPC):
            et = const.tile([128, 4 * N], F32R, tag=f"edge{bb}")
            for c in range(4):
                nc.sync.dma_start(
                    et[:, c * N : (c + 1) * N],
                    edgeT_in[bb, c * 128 : (c + 1) * 128, :],
                )
            edge_sb.append(et)

        def emit_vstage(bb, y, widx):
            """psum v-tile: [x@(c W1) | x@(c W2)] per 128-node chunk."""
            pvt = pv.tile([128, 4 * 256], F32, tag=f"pv{bb}")
            for c in range(4):
                nc.tensor.matmul(
                    pvt[:, c * 256 : (c + 1) * 256],
                    lhsT=y[:, c * 128 : (c + 1) * 128],
                    rhs=wcat_slice(widx),
                    start=True,
                    stop=True,
                )
            return pvt

        def emit_vcopy(bb, pvt):
            vt = vpool.tile([128, N], F32R, tag=f"v{bb}")
            dst = vt[:].rearrange("p (c e) -> p c e", c=4)
            src = pvt[:].rearrange("p (c w) -> p c w", c=4)[:, :, 0:128]
            nc.scalar.activation(dst, src, ACTF.Copy)
            return vt

        def emit_zstage(bb, y, vt, widx, seed_sb):
            """psum z = [seed] + (E @ v)^T + (y^T @ c W2)^T."""
            pzt = pz.tile([128, N], F32, tag=f"pz{bb}")
            if seed_sb is not None:
                nc.tensor.matmul(
                    pzt[:], lhsT=ident[:], rhs=seed_sb[:], start=True, stop=False
                )
            nc.tensor.matmul(
                pzt[:],
                lhsT=w2_slice(widx),
                rhs=y[:],
                start=(seed_sb is None),
                stop=False,
            )
            for c in range(4):
                nc.tensor.matmul(
                    pzt[:],
                    lhsT=vt[:, c * 128 : (c + 1) * 128],
                    rhs=edge_sb[bb][:, c * N : (c + 1) * N],
                    start=False,
                    stop=(c == 3),
                )
            return pzt

        loop_ctx = tc.For_i(0, repeat, 1) if repeat > 1 else None
        if loop_ctx is not None:
            ctx.enter_context(loop_ctx)
        for rep in range(1):
            x_cur = []
            for bb in range(BPC):
                x0 = state.tile([D, N], F32R, tag=f"x{bb}")
                nc.sync.dma_start(x0[:], nodeT_in[bb])
                x_cur.append(x0)

            for t in range(T - 1):
                dt = float(dts[t])
                di = dt_vals.index(dt)
                w_half = 1 + 2 * di      # (dt/2) * [W1|W2]
                w_full_dt = 2 + 2 * di   # dt * [W1|W2]
                ks = [[None] * 4 for _ in range(BPC)]
                acc = [None] * BPC
                z1_sb = [None] * BPC
                for e in range(4):
                    widx = (0, w_half, w_half, w_full_dt)[e]
                    ys = [
                        x_cur[bb] if e == 0 else ks[bb][e - 1] for bb in range(BPC)
                    ]
                    pvts = [emit_vstage(bb, ys[bb], widx) for bb in range(BPC)]
                    vts = [emit_vcopy(bb, pvts[bb]) for bb in range(BPC)]
                    pzts = [
                        emit_zstage(
                            bb, ys[bb], vts[bb], widx,
                            None if e == 0 else z1_sb[bb],
                        )
                        for bb in range(BPC)
                    ]
                    for bb in range(BPC):
                        k = kpool.tile([D, N], F32R, tag=f"k{e}_{bb}")
                        nc.scalar.activation(k[:], pzts[bb][:], ACTF.Tanh, bias=bias[:])
                        ks[bb][e] = k
                    if e == 0:
                        for bb in range(BPC):
                            z1 = zpool.tile([D, N], F32R, tag=f"z1_{bb}")
                            nc.vector.tensor_copy(z1[:], pzts[bb][:])
                            z1_sb[bb] = z1
                    # RK4 combine chain, one link per eval (off critical path)
                    cscale = (dt / 6.0, dt / 3.0, dt / 3.0, dt / 6.0)[e]
                    for bb in range(BPC):
                        prev = x_cur[bb] if e == 0 else acc[bb]
                        if e < 3:
                            a = tmp.tile([D, N], F32, tag=f"a{bb}")
                            nc.vector.scalar_tensor_tensor(
                                a[:], ks[bb][e][:], cscale, prev[:], ALU.mult, ALU.add
                            )
                            acc[bb] = a
                        else:
                            x_new = state.tile([D, N], F32R, tag=f"x{bb}")
                            nc.vector.scalar_tensor_tensor(
                                x_new[:], ks[bb][e][:], cscale, prev[:],
                                ALU.mult, ALU.add,
                            )
                            nc.sync.dma_start(out_t[t, bb], x_new[:])
                            x_cur[bb] = x_new


def round_f32r(x):
    """Round fp32 values to the fp32r subset (11 explicit mantissa bits,
    low 12 bits zero) with round-to-nearest-even — matches what the PE
    consumes in fp32r mode, so host-side rounding keeps hardware exact."""
    u = np.ascontiguousarray(x, dtype=np.float32).view(np.uint32)
    u = (u + 0x7FF + ((u >> 12) & 1)) & np.uint32(0xFFFFF000)
    return u.view(np.float32)


def make_in_maps(node, edge, time_steps, W1, W2, b):
    dts = np.asarray(time_steps, np.float32)
    dts = dts[1:] - dts[:-1]
    dt_vals = sorted({float(d) for d in dts})
    wcat = np.concatenate([W1, W2], axis=1).astype(np.float32)
    wcats = [wcat]
    w2s = [W2.astype(np.float32)]
    for dv in dt_vals:
        wcats.append(wcat * (dv / 2))
        wcats.append(wcat * dv)
        w2s.append(W2 * (dv / 2))
        w2s.append(W2 * dv)
    wcats = round_f32r(np.stack(wcats))
    w2s = round_f32r(np.stack(w2s))
    ident = round_f32r(np.eye(D, dtype=np.float32))
    bc = np.ascontiguousarray(np.reshape(b, (D, 1)), dtype=np.float32)
    in_maps = []
    for core in range(NCORES):
        sl = slice(core * BPC, (core + 1) * BPC)
        in_maps.append(
            {
                "nodeT": round_f32r(node[sl].transpose(0, 2, 1)),
                "edgeT": round_f32r(edge[sl].transpose(0, 2, 1)),
                "wcats": wcats,
                "w2s": w2s,
                "ident": ident,
                "bvec": bc,
            }
        )
    return in_maps


LAST_RESULT = None


def kernel(node, edge, time_steps, W1, W2, b, trace=False):
    node = np.asarray(node, dtype=np.float32)
    edge = np.asarray(edge, dtype=np.float32)
    time_steps = np.asarray(time_steps, dtype=np.float32)
    W1 = np.asarray(W1, dtype=np.float32)
    W2 = np.asarray(W2, dtype=np.float32)
    b = np.asarray(b, dtype=np.float32)

    dts = time_steps[1:] - time_steps[:-1]
    nc = build_program(dts)
    in_maps = make_in_maps(node, edge, time_steps, W1, W2, b)
    res = bass_utils.run_bass_kernel_spmd(
        nc, in_maps, core_ids=list(range(NCORES)), trace=trace
    )
    global LAST_RESULT
    LAST_RESULT = res
    outs = [res.results[c]["out"] for c in range(NCORES)]  # [T-1, BPC, D, N]
    full = np.concatenate(outs, axis=1)  # [T-1, B, D, N]
    pred = np.empty((T, B, N, D), dtype=np.float32)
    pred[0] = node
    pred[1:] = full.transpose(0, 1, 3, 2)
    return pred


# revision 9
# speedup vs baseline: 173.7406x; 1.0061x over previous
"""TRN2 Bass/Tile kernel: graph neural ODE integrated with RK4.

Computes pred_y[t] for t=0..19 where
    dx/dt = f(x) = tanh((edge @ x) @ W1 + x @ W2 + b)
via 19 RK4 steps from x0 = node, data-parallel over the batch axis:
16 batches sharded 2-per-core across 8 NeuronCores (SPMD, no collectives).

Layout strategy (per core, per batch):
  - State lives TRANSPOSED in SBUF: xT[d, i]  (feature on partitions,
    512 nodes on the free axis).
  - v-stage:  v = y @ W1 in natural [node, feat] layout via
        matmul(lhsT=yT[:, c*128:(c+1)*128], rhs=[W1|W2])  -> psum[j, 256]
    (the widened [W1|W2] moving operand keeps the fp32r fast path,
    which needs a moving free dim >= 256; the W2 half is discarded).
  - z-stage:  zT[e, i] = (edge @ v)^T + (y @ W2)^T accumulated in one
    PSUM bank.
  - Z-LINEARITY: z() is linear, so the RK4 intermediate states never
    materialize:  z(x + c*k) = Z1 + c*Z(k).  Evals 2-4 run the v/z
    stages on k_{i-1} with pre-scaled weights (c*[W1|W2], c*W2) and
    accumulate on top of a seed matmul(identity, Z1_sb).
  - tanh on ScalarE straight out of PSUM with per-partition bias b.
  - RK4 combine is a chain of fused scalar_tensor_tensor ops
        a1 = x + dt/6 k1; a2 = a1 + dt/3 k2; a3 = a2 + dt/3 k3;
        x_new = a3 + dt/6 k4
    where a_i runs as soon as k_i exists (off the critical path).
  - edge is consumed transposed (edgeT[j, i]); the host pre-transposes it
    (free), and the per-step outputs are written transposed [d, i] and
    un-transposed on the host (also free).

All matmuls run in float32r (fp32 rounded to 11 explicit mantissa bits;
the PE runs 1 cycle/row for moving dims >= 256).  Values feeding matmuls
are produced as float32r (DMA of host-pre-rounded data, ACT tanh/copy
outputs, DVE STT outputs), which is what the walrus verifier requires.
"""

import numpy as np

import concourse.bass as bass
import concourse.tile as tile
from concourse import bacc, mybir
from concourse import bass_utils

B, N, D, T = 16, 512, 128, 20
NCORES = 8
BPC = B // NCORES  # batches per core

F32 = mybir.dt.float32
F32R = mybir.dt.float32r
ALU = mybir.AluOpType
ACTF = mybir.ActivationFunctionType


def build_program(dts, repeat=1):
    """Build the SPMD Bass program (identical on all cores).

    repeat > 1 re-runs the whole integration from x0 that many times
    (timing runs only; the output stays that of the final pass).
    """
    nc = bacc.Bacc(
        "TRN2",
        target_bir_lowering=False,
        debug=False,
        num_devices=NCORES,
    )
    dt_vals = sorted({float(d) for d in dts})
    nodeT_in = nc.dram_tensor("nodeT", [BPC, D, N], F32R, kind="ExternalInput").ap()
    edgeT_in = nc.dram_tensor("edgeT", [BPC, N, N], F32R, kind="ExternalInput").ap()
    # per distinct dt: [W1|W2], c/2*[W1|W2], c*[W1|W2] are slices of wcats
    wcats_in = nc.dram_tensor(
        "wcats", [1 + 2 * len(dt_vals), D, 2 * D], F32R, kind="ExternalInput"
    ).ap()
    w2s_in = nc.dram_tensor(
        "w2s", [1 + 2 * len(dt_vals), D, D], F32R, kind="ExternalInput"
    ).ap()
    ident_in = nc.dram_tensor("ident", [D, D], F32R, kind="ExternalInput").ap()
    b_in = nc.dram_tensor("bvec", [D, 1], F32, kind="ExternalInput").ap()
    out_t = nc.dram_tensor("out", [T - 1, BPC, D, N], F32R, kind="ExternalOutput").ap()

    with tile.TileContext(nc) as tc:
        _emit(
            tc, nodeT_in, edgeT_in, wcats_in, w2s_in, ident_in, b_in, out_t,
            dts, dt_vals, repeat,
        )
    nc.compile()
    return nc


def _emit(tc, nodeT_in, edgeT_in, wcats_in, w2s_in, ident_in, b_in, out_t,
          dts, dt_vals, repeat):
    from contextlib import ExitStack

    nc = tc.nc
    nw = 1 + 2 * len(dt_vals)
    with ExitStack() as ctx:
        const = ctx.enter_context(tc.tile_pool(name="const", bufs=1))
        state = ctx.enter_context(tc.tile_pool(name="state", bufs=2))
        kpool = ctx.enter_context(tc.tile_pool(name="k", bufs=2))
        vpool = ctx.enter_context(tc.tile_pool(name="v", bufs=3))
        zpool = ctx.enter_context(tc.tile_pool(name="z1", bufs=2))
        tmp = ctx.enter_context(tc.tile_pool(name="tmp", bufs=2))
        pv = ctx.enter_context(tc.tile_pool(name="pv", bufs=1, space="PSUM"))
        pz = ctx.enter_context(tc.tile_pool(name="pz", bufs=2, space="PSUM"))

        wcats = const.tile([D, nw * 2 * D], F32R, tag="wcats")
        for w in range(nw):
            nc.sync.dma_start(wcats[:, w * 2 * D : (w + 1) * 2 * D], wcats_in[w])
        w2s = const.tile([D, nw * D], F32R, tag="w2s")
        for w in range(nw):
            nc.sync.dma_start(w2s[:, w * D : (w + 1) * D], w2s_in[w])
        ident = const.tile([D, D], F32R, tag="ident")
        nc.sync.dma_start(ident[:], ident_in)
        bias = const.tile([D, 1], F32, tag="bias")
        nc.sync.dma_start(bias[:], b_in)

        def wcat_slice(idx):
            return wcats[:, idx * 2 * D : (idx + 1) * 2 * D]

        def w2_slice(idx):
            return w2s[:, idx * D : (idx + 1) * D]

        edge_sb = []
        for bb in range(BPC):
            et = const.tile([128, 4 * N], F32R, tag=f"edge{bb}")
            for c in range(4):
                nc.sync.dma_start(
                    et[:, c * N : (c + 1) * N],
                    edgeT_in[bb, c * 128 : (c + 1) * 128, :],
                )
            edge_sb.append(et)

        def emit_vstage(bb, y, widx):
            """psum v-tile: [x@(c W1) | x@(c W2)] per 128-node chunk."""
            pvt = pv.tile([128, 4 * 256], F32, tag=f"pv{bb}")
            for c in range(4):
                nc.tensor.matmul(
                    pvt[:, c * 256 : (c + 1) * 256],
                    lhsT=y[:, c * 128 : (c + 1) * 128],
                    rhs=wcat_slice(widx),
                    start=True,
                    stop=True,
                )
            return pvt

        def emit_vcopy(bb, pvt):
            vt = vpool.tile([128, N], F32R, tag=f"v{bb}")
            dst = vt[:].rearrange("p (c e) -> p c e", c=4)
            src = pvt[:].rearrange("p (c w) -> p c w", c=4)[:, :, 0:128]
            nc.scalar.activation(dst, src, ACTF.Copy)
            return vt

        def emit_zstage(bb, y, vt, widx, seed_sb):
            """psum z = [seed] + (E @ v)^T + (y^T @ c W2)^T."""
            pzt = pz.tile([128, N], F32, tag=f"pz{bb}")
            if seed_sb is not None:
                nc.tensor.matmul(
                    pzt[:], lhsT=ident[:], rhs=seed_sb[:], start=True, stop=False
                )
            nc.tensor.matmul(
                pzt[:],
                lhsT=w2_slice(widx),
                rhs=y[:],
                start=(seed_sb is None),
                stop=False,
            )
            for c in range(4):
                nc.tensor.matmul(
                    pzt[:],
                    lhsT=vt[:, c * 128 : (c + 1) * 128],
                    rhs=edge_sb[bb][:, c * N : (c + 1) * N],
                    start=False,
                    stop=(c == 3),
                )
            return pzt

        loop_ctx = tc.For_i(0, repeat, 1) if repeat > 1 else None
        if loop_ctx is not None:
            ctx.enter_context(loop_ctx)
        for rep in range(1):
            x_cur = []
            for bb in range(BPC):
                x0 = state.tile([D, N], F32R, tag=f"x{bb}")
                nc.sync.dma_start(x0[:], nodeT_in[bb])
                x_cur.append(x0)

            for t in range(T - 1):
                dt = float(dts[t])
                di = dt_vals.index(dt)
                w_half = 1 + 2 * di      # (dt/2) * [W1|W2]
                w_full_dt = 2 + 2 * di   # dt * [W1|W2]
                ks = [[None] * 4 for _ in range(BPC)]
                acc = [None] * BPC
                z1_sb = [None] * BPC
                for e in range(4):
                    widx = (0, w_half, w_half, w_full_dt)[e]
                    ys = [
                        x_cur[bb] if e == 0 else ks[bb][e - 1] for bb in range(BPC)
                    ]
                    pvts = [emit_vstage(bb, ys[bb], widx) for bb in range(BPC)]
                    vts = [emit_vcopy(bb, pvts[bb]) for bb in range(BPC)]
                    pzts = [
                        emit_zstage(
                            bb, ys[bb], vts[bb], widx,
                            None if e == 0 else z1_sb[bb],
                        )
                        for bb in range(BPC)
                    ]
                    for bb in range(BPC):
                        k = kpool.tile([D, N], F32R, tag=f"k{e}_{bb}")
                        nc.scalar.activation(k[:], pzts[bb][:], ACTF.Tanh, bias=bias[:])
                        ks[bb][e] = k
                    if e == 0:
                        for bb in range(BPC):
                            z1 = zpool.tile([D, N], F32R, tag=f"z1_{bb}")
                            nc.vector.tensor_copy(z1[:], pzts[bb][:])
                            z1_sb[bb] = z1
                    # RK4 combine chain, one link per eval (off critical path)
                    cscale = (dt / 6.0, dt / 3.0, dt / 3.0, dt / 6.0)[e]
                    for bb in range(BPC):
                        prev = x_cur[bb] if e == 0 else acc[bb]
                        if e < 3:
                            a = tmp.tile([D, N], F32, tag=f"a{bb}")
                            nc.vector.scalar_tensor_tensor(
                                a[:], ks[bb][e][:], cscale, prev[:], ALU.mult, ALU.add
                            )
                            acc[bb] = a
                        else:
                            x_new = state.tile([D, N], F32R, tag=f"x{bb}")
                            nc.vector.scalar_tensor_tensor(
                                x_new[:], ks[bb][e][:], cscale, prev[:],
                                ALU.mult, ALU.add,
                            )
                            nc.sync.dma_start(out_t[t, bb], x_new[:])
                            x_cur[bb] = x_new


def round_f32r(x):
    """Round fp32 values to the fp32r subset (11 explicit mantissa bits,
    low 12 bits zero) with round-to-nearest-even — matches what the PE
    consumes in fp32r mode, so host-side rounding keeps hardware exact."""
    u = np.ascontiguousarray(x, dtype=np.float32).view(np.uint32)
    u = (u + 0x7FF + ((u >> 12) & 1)) & np.uint32(0xFFFFF000)
    return u.view(np.float32)


def make_in_maps(node, edge, time_steps, W1, W2, b):
    dts = np.asarray(time_steps, np.float32)
    dts = dts[1:] - dts[:-1]
    dt_vals = sorted({float(d) for d in dts})
    wcat = np.concatenate([W1, W2], axis=1).astype(np.float32)
    wcats = [wcat]
    w2s = [W2.astype(np.float32)]
    for dv in dt_vals:
        wcats.append(wcat * (dv / 2))
        wcats.append(wcat * dv)
        w2s.append(W2 * (dv / 2))
        w2s.append(W2 * dv)
    wcats = round_f32r(np.stack(wcats))
    w2s = round_f32r(np.stack(w2s))
    ident = round_f32r(np.eye(D, dtype=np.float32))
    bc = np.ascontiguousarray(np.reshape(b, (D, 1)), dtype=np.float32)
    in_maps = []
    for core in range(NCORES):
        sl = slice(core * BPC, (core + 1) * BPC)
        in_maps.append(
            {
                "nodeT": round_f32r(node[sl].transpose(0, 2, 1)),
                "edgeT": round_f32r(edge[sl].transpose(0, 2, 1)),
                "wcats": wcats,
                "w2s": w2s,
                "ident": ident,
                "bvec": bc,
            }
        )
    return in_maps


LAST_RESULT = None


def kernel(node, edge, time_steps, W1, W2, b, trace=False):
    node = np.asarray(node, dtype=np.float32)
    edge = np.asarray(edge, dtype=np.float32)
    time_steps = np.asarray(time_steps, dtype=np.float32)
    W1 = np.asarray(W1, dtype=np.float32)
    W2 = np.asarray(W2, dtype=np.float32)
    b = np.asarray(b, dtype=np.float32)

    dts = time_steps[1:] - time_steps[:-1]
    nc = build_program(dts)
    in_maps = make_in_maps(node, edge, time_steps, W1, W2, b)
    res = bass_utils.run_bass_kernel_spmd(
        nc, in_maps, core_ids=list(range(NCORES)), trace=trace
    )
    global LAST_RESULT
    LAST_RESULT = res
    outs = [res.results[c]["out"] for c in range(NCORES)]  # [T-1, BPC, D, N]
    full = np.concatenate(outs, axis=1)  # [T-1, B, D, N]
    pred = np.empty((T, B, N, D), dtype=np.float32)
    pred[0] = node
    pred[1:] = full.transpose(0, 1, 3, 2)
    return pred
